# revision 41
# baseline (speedup 1.0000x reference)
import sys, os
for _p in ("/opt/trn_rl_repo",):
    if _p not in sys.path:
        sys.path.append(_p)

import numpy as np
import ml_dtypes
from contextlib import ExitStack

import concourse.bass as bass
import concourse.bacc as bacc
import concourse.tile as tile
from concourse import mybir
from concourse.bass_utils import run_bass_kernel_spmd

F32 = mybir.dt.float32
BF16 = mybir.dt.bfloat16
BF_NP = ml_dtypes.bfloat16

DIM = 256
HEADS = 8
DIM_HEAD = 64
SLICE_NUM = 64
INNER = HEADS * DIM_HEAD  # 512
B, N = 4, 32768
NCORES = 8
NSHARD = N // 2  # 16384 tokens per core
P = 128
EXPF = mybir.ActivationFunctionType.Exp


def build_program(nshard, dbg=False):
    NT = nshard // P
    assert NT % 2 == 0
    nc = bacc.Bacc("TRN2", target_bir_lowering=False, debug=False,
                   num_devices=NCORES)
    if dbg:
        dbg_pooled = nc.dram_tensor("dbg_pooled", [P, 4, 130], F32,
                                    kind="ExternalOutput").ap()
        dbg_m2 = nc.dram_tensor("dbg_m2", [P, 4, DIM], BF16,
                                kind="ExternalOutput").ap()
        dbg_wT = nc.dram_tensor("dbg_wT", [P, 4, nshard], BF16,
                                kind="ExternalOutput").ap()
    xT_h = nc.dram_tensor("xT", [DIM, nshard], BF16, kind="ExternalInput")
    wfxT = nc.dram_tensor("wfxT", [DIM, INNER], BF16, kind="ExternalInput").ap()
    wlgT = nc.dram_tensor("wlgT", [DIM, INNER], BF16, kind="ExternalInput").ap()
    blg = nc.dram_tensor("blg", [1, INNER], BF16, kind="ExternalInput").ap()
    onesb = nc.dram_tensor("onesb", [1, P], BF16, kind="ExternalInput").ap()
    bfxb = nc.dram_tensor("bfxb", [P, 4, 64], F32, kind="ExternalInput").ap()
    wqT = nc.dram_tensor("wqT", [64, 64], F32, kind="ExternalInput").ap()
    wkT = nc.dram_tensor("wkT", [64, 64], F32, kind="ExternalInput").ap()
    wvT = nc.dram_tensor("wvT", [64, 64], F32, kind="ExternalInput").ap()
    woT = nc.dram_tensor("woT", [64, HEADS, DIM], F32, kind="ExternalInput").ap()
    bout8b = nc.dram_tensor("bout8b", [P, 2, DIM], F32,
                            kind="ExternalInput").ap()
    idf32 = nc.dram_tensor("idf32", [P, P], F32, kind="ExternalInput").ap()
    out_h = nc.dram_tensor("out", [nshard, DIM], F32, kind="ExternalOutput")
    out_ap = out_h.ap()

    with tile.TileContext(nc) as tc, ExitStack() as ctx:
        cpool = ctx.enter_context(tc.tile_pool(name="consts", bufs=1))
        big = ctx.enter_context(tc.tile_pool(name="big", bufs=1))

        # big weights on the scalar queue so x tiles start on sync at once
        wfx_sb = cpool.tile([P, 2, INNER], BF16)
        wlg_sb = cpool.tile([P, 2, INNER], BF16)
        for c in range(2):
            nc.scalar.dma_start(wfx_sb[:, c, :], wfxT[c * P:(c + 1) * P, :])
            nc.scalar.dma_start(wlg_sb[:, c, :], wlgT[c * P:(c + 1) * P, :])
        blg_sb = cpool.tile([1, INNER], BF16)
        nc.scalar.dma_start(blg_sb[:], blg[:])
        ones1 = cpool.tile([1, P], BF16)
        nc.scalar.dma_start(ones1[:], onesb[:])
        bfx_sb = cpool.tile([P, 4, 64], F32)
        nc.scalar.dma_start(bfx_sb[:], bfxb[:])
        wq_sb = cpool.tile([64, 64], F32)
        wk_sb = cpool.tile([64, 64], F32)
        wv_sb = cpool.tile([64, 64], F32)
        nc.scalar.dma_start(wq_sb[:], wqT[:])
        nc.scalar.dma_start(wk_sb[:], wkT[:])
        nc.scalar.dma_start(wv_sb[:], wvT[:])
        wo_sb = cpool.tile([64, HEADS, DIM], F32)
        nc.scalar.dma_start(wo_sb[:], woT[:])
        bout82_sb = cpool.tile([P, 2, DIM], F32)
        nc.scalar.dma_start(bout82_sb[:], bout8b[:])
        idf_sb = cpool.tile([P, P], F32)
        nc.scalar.dma_start(idf_sb[:], idf32[:])

        # persistent across phases
        # transposed slice weights, blocked: [g, t4, (t%4)*4+c, tok]
        wT_sb = big.tile([P, nshard // (4 * P), 16, P], BF16)
        pooled_sb = big.tile([P, 4, 130], F32)   # after allreduce
        m2_sb = big.tile([P, 4, DIM], BF16)      # out_slice @ WoutT per hg
        # manual 6-slot fx staging; ones cols preset once (norm columns)
        FXS = 6
        fx2_sb = big.tile([P, FXS, 4, 130], BF16)
        nc.vector.memset(fx2_sb[:, :, :, 128:130], 1.0)
        # 12-slot w staging ring, consumed by pools + 4-wide batched transpose
        WS = 12
        w8_sb = big.tile([P, WS, HEADS, SLICE_NUM], BF16)

        # ---------------- pass 1 ----------------
        # software-pipelined: pool matmuls + wT transpose for sub-tile t are
        # emitted DLY iterations late so the PE/sync queues never head-of-line
        # block on the exp->reduce->recip->mul chain.
        DLY = 4
        XB = 4  # sub-tiles per x DMA
        with tc.tile_pool(name="xp", bufs=3) as xpool, \
             tc.tile_pool(name="sp", bufs=8) as spool, \
             tc.tile_pool(name="fxps", bufs=3, space="PSUM") as fxps, \
             tc.tile_pool(name="lgps", bufs=3, space="PSUM") as lgps, \
             tc.tile_pool(name="poolps", bufs=1, space="PSUM") as poolps:
            # two accumulators per bank; start=True resets per-address, so
            # disjoint column ranges in one bank are safe
            pool_ps = [poolps.tile([P, 2, 130], F32, name=f"pool_ps{i}")
                       for i in range(2)]

            def emit_late(u):
                for q in range(4):
                    nc.tensor.matmul(pool_ps[q // 2][:, q % 2, :],
                                     w8_sb[:, u % WS, 2 * q:2 * q + 2, :],
                                     fx2_sb[:, u % FXS, q, :],
                                     start=(u == 0), stop=(u == NT - 1))
                if u % 4 == 3:
                    # one blocked DMA transpose for 4 sub-tiles:
                    # wT[g, (t',c), tok] = w[tok, (t',c)*128+g]
                    b = u // 4
                    s0 = (b % (WS // 4)) * 4
                    nc.sync.dma_start_transpose(
                        wT_sb[:, b, :, :], w8_sb[:, s0:s0 + 4, :, :])

            # warm the PE clock before the steady-state stream (scratch
            # writes into pool_ps[0]; the real accumulation's start=True at
            # t==0 overwrites them)
            for _ in range(15):
                nc.tensor.matmul(pool_ps[0][:], wfx_sb[:, 0, 0:128],
                                 wfx_sb[:, 0, 0:260], start=True, stop=True)

            for t in range(NT):
                if t % XB == 0:
                    xt = xpool.tile([P, 2, XB * P], BF16)
                    src = bass.AP(xT_h, t * P,
                                  [[nshard, P], [P * nshard, 2], [1, XB * P]])
                    nc.sync.dma_start(xt[:], src)
                s = t % XB
                xa = xt[:, 0, s * P:(s + 1) * P]
                xb = xt[:, 1, s * P:(s + 1) * P]
                fxp = fxps.tile([P, 4, P], F32)
                nc.tensor.matmul(fxp[:], xa, wfx_sb[:, 0, :],
                                 start=True, stop=False)
                nc.tensor.matmul(fxp[:], xb, wfx_sb[:, 1, :],
                                 start=False, stop=True)
                lgp = lgps.tile([P, HEADS, SLICE_NUM], F32)
                nc.tensor.matmul(lgp[:], ones1[:], blg_sb[:],
                                 start=True, stop=False)
                nc.tensor.matmul(lgp[:], xa, wlg_sb[:, 0, :],
                                 start=False, stop=False)
                nc.tensor.matmul(lgp[:], xb, wlg_sb[:, 1, :],
                                 start=False, stop=True)
                # softmax over slices (bounded logits: skip max-sub)
                nc.scalar.copy(fx2_sb[:, t % FXS, 0:2, 0:128], fxp[:, 0:2, :])
                e_t = spool.tile([P, HEADS, SLICE_NUM], BF16)
                nc.scalar.activation(e_t[:], lgp[:], EXPF)
                s_t = spool.tile([P, HEADS], F32)
                nc.vector.tensor_copy(fx2_sb[:, t % FXS, 2:4, 0:128],
                                      fxp[:, 2:4, :])
                nc.vector.tensor_reduce(s_t[:], e_t[:],
                                        axis=mybir.AxisListType.X,
                                        op=mybir.AluOpType.add)
                r_t = spool.tile([P, HEADS], F32)
                nc.vector.reciprocal(r_t[:], s_t[:])
                nc.gpsimd.tensor_mul(
                    w8_sb[:, t % WS, :, :], e_t[:],
                    r_t[:, :, None].to_broadcast([P, HEADS, SLICE_NUM]))
                if t >= DLY:
                    emit_late(t - DLY)
            for u in range(NT - DLY, NT):
                emit_late(u)

            # -------- allreduce pooled sums over the token-half pair --------
            with tc.tile_pool(name="ccdram", bufs=1, space="DRAM") as dpool:
                b_in = dpool.tile([P, 4, 130], BF16)
                b_out = dpool.tile([P, 4, 130], BF16)
                pre_sb = big.tile([P, 4, 130], BF16)
                nc.scalar.copy(pre_sb[:, 0:2, :], pool_ps[0][:])
                nc.vector.tensor_copy(pre_sb[:, 2:4, :], pool_ps[1][:])
                nc.sync.dma_start(b_in[:], pre_sb[:])
                nc.gpsimd.collective_compute(
                    "AllReduce", mybir.AluOpType.add,
                    replica_groups=[[0, 1], [2, 3], [4, 5], [6, 7]],
                    ins=[b_in.opt()], outs=[b_out.opt()])
                pooled_bf = big.tile([P, 4, 130], BF16)
                nc.sync.dma_start(pooled_bf[:], b_out[:])
                nc.vector.tensor_copy(pooled_sb[:], pooled_bf[:])

        # ------- tiny slice attention, stage-batched across head pairs ----
        with tc.tile_pool(name="mps", bufs=1, space="PSUM") as mps, \
             tc.tile_pool(name="msb", bufs=1) as msb:
            # slice tokens: st = (diag(pooled) + bfx*norm) / (norm + 1e-5)
            norm = pooled_sb[:, :, 128:129]
            nrm = msb.tile([P, 4], F32)
            nc.vector.tensor_scalar_add(nrm[:], pooled_sb[:, :, 128], 1e-5)
            rho = msb.tile([P, 4], F32)
            nc.vector.reciprocal(rho[:], nrm[:])
            tmp = msb.tile([P, 4, 64], F32)
            nc.vector.tensor_mul(tmp[:], bfx_sb[:],
                                 norm.to_broadcast([P, 4, 64]))
            for j in range(2):
                sl = slice(j * 64, (j + 1) * 64)
                nc.vector.tensor_add(tmp[sl, :, :], tmp[sl, :, :],
                                     pooled_sb[sl, :, j * 64:j * 64 + 64])
            st = msb.tile([P, 4, 64], F32)
            nc.vector.tensor_mul(st[:], tmp[:],
                                 rho[:, :, None].to_broadcast([P, 4, 64]))
            # stT[c, q4, j*64+g] = st[j*64+g, q4, c]
            pA = mps.tile([64, 4, P], F32, name="pA")  # stT / aT / osT
            pB = mps.tile([64, 4, P], F32, name="pB")  # q
            pC = mps.tile([64, 4, P], F32, name="pC")  # k
            pD = mps.tile([P, 4, 64], F32, name="pD")  # L / os
            pV0 = mps.tile([64, 4, 64], F32, name="pV0")
            pV1 = mps.tile([64, 4, 64], F32, name="pV1")
            pE = mps.tile([P, 2, DIM], F32, name="pE")
            pF = mps.tile([P, 2, DIM], F32, name="pF")
            for q4 in range(4):
                nc.tensor.transpose(pA[:, q4, :], st[:, q4, :], idf_sb[:])
            stT = msb.tile([64, 4, P], F32)
            nc.scalar.copy(stT[:], pA[:])
            for q4 in range(4):
                nc.tensor.matmul(pB[:, q4, :], wq_sb[:], stT[:, q4, :],
                                 start=True, stop=True)
                nc.tensor.matmul(pC[:, q4, :], wk_sb[:], stT[:, q4, :],
                                 start=True, stop=True)
            qq = msb.tile([64, 4, P], F32)
            nc.scalar.copy(qq[:], pB[:])
            kk = msb.tile([64, 4, P], F32)
            nc.vector.tensor_copy(kk[:], pC[:])
            # v[g', o] per (q4, j): lhsT = stT j-half, rhs = Wv
            for q4 in range(4):
                nc.tensor.matmul(pV0[:, q4, :], stT[:, q4, 0:64], wv_sb[:],
                                 start=True, stop=True)
                nc.tensor.matmul(pV1[:, q4, :], stT[:, q4, 64:128], wv_sb[:],
                                 start=True, stop=True)
            v0 = msb.tile([64, 4, 64], F32)
            nc.scalar.copy(v0[:], pV0[:])
            v1 = msb.tile([64, 4, 64], F32)
            nc.vector.tensor_copy(v1[:], pV1[:])
            # attention logits per (q4, head j)
            for q4 in range(4):
                for j in range(2):
                    sl = slice(j * 64, (j + 1) * 64)
                    nc.tensor.matmul(pD[sl, q4, :], qq[:, q4, sl],
                                     kk[:, q4, sl], start=True, stop=True)
            ea = msb.tile([P, 4, 64], F32)
            nc.scalar.activation(ea[:], pD[:], EXPF)
            sr = msb.tile([P, 4], F32)
            nc.vector.tensor_reduce(sr[:], ea[:], axis=mybir.AxisListType.X,
                                    op=mybir.AluOpType.add)
            rha = msb.tile([P, 4], F32)
            nc.vector.reciprocal(rha[:], sr[:])
            attn = msb.tile([P, 4, 64], F32)
            nc.vector.tensor_mul(attn[:], ea[:],
                                 rha[:, :, None].to_broadcast([P, 4, 64]))
            # aT[g, q4, j*64+g'] = attn[j*64+g', q4, g]
            for q4 in range(4):
                nc.tensor.transpose(pA[:, q4, :], attn[:, q4, :], idf_sb[:])
            aT = msb.tile([64, 4, P], F32)
            nc.scalar.copy(aT[:], pA[:])
            # os = attn @ v  (reuses pD)
            for q4 in range(4):
                for j in range(2):
                    sl = slice(j * 64, (j + 1) * 64)
                    nc.tensor.matmul(pD[sl, q4, :], aT[:, q4, sl],
                                     (v0 if j == 0 else v1)[:, q4, :],
                                     start=True, stop=True)
            os_sb = msb.tile([P, 4, 64], F32)
            nc.scalar.copy(os_sb[:], pD[:])
            for q4 in range(4):
                nc.tensor.transpose(pA[:, q4, :], os_sb[:, q4, :], idf_sb[:])
            osT = msb.tile([64, 4, P], F32)
            nc.scalar.copy(osT[:], pA[:])
            # m2 = osT @ WoutT per (q4, j), + bout/8 fold
            for q4 in range(4):
                pm = pE if q4 < 2 else pF
                for j in range(2):
                    sl = slice(j * 64, (j + 1) * 64)
                    nc.tensor.matmul(pm[sl, q4 % 2, :], osT[:, q4, sl],
                                     wo_sb[:, 2 * q4 + j, :],
                                     start=True, stop=True)
            nc.vector.tensor_add(m2_sb[:, 0:2, :], pE[:], bout82_sb[:])
            nc.vector.tensor_add(m2_sb[:, 2:4, :], pF[:], bout82_sb[:])

        # ---------------- pass 2: unpool + output proj ----------------
        with tc.tile_pool(name="p2ps", bufs=8, space="PSUM") as p2ps, \
             tc.tile_pool(name="p2sb", bufs=5) as p2sb:
            for t4 in range(NT // 4):
                ob4 = p2sb.tile([P, 4, DIM], F32)
                for k in range(4):
                    t = 4 * t4 + k
                    op = p2ps.tile([P, DIM], F32)
                    for c in range(4):
                        nc.tensor.matmul(
                            op[:], wT_sb[:, t4, k * 4 + c, :],
                            m2_sb[:, c, :],
                            start=(c == 0), stop=(c == 3))
                    if k % 2 == 0:
                        nc.vector.tensor_copy(ob4[:, k, :], op[:])
                    else:
                        nc.scalar.copy(ob4[:, k, :], op[:])
                # one quad DMA: dst rows t4*512 + k*128 + p
                dst = bass.AP(out_h, t4 * 4 * P * DIM,
                              [[DIM, P], [P * DIM, 4], [1, DIM]])
                nc.sync.dma_start(dst, ob4[:])
        if dbg:
            nc.sync.dma_start(dbg_pooled[:], pooled_sb[:])
            nc.sync.dma_start(dbg_m2[:], m2_sb[:])
            nc.sync.dma_start(dbg_wT[:], wT_sb[:])
    nc.compile()
    return nc


def _bfx_pair(bfx):
    bfx2 = bfx.reshape(HEADS, DIM_HEAD)
    out = np.empty((P, 4, 64), np.float32)
    for q4 in range(4):
        for j in range(2):
            out[j * 64:(j + 1) * 64, q4, :] = bfx2[2 * q4 + j]
    return out


def prep_weights(inputs):
    f32 = np.float32
    Wfx = np.asarray(inputs["Wfx"], f32)
    bfx = np.asarray(inputs["bfx"], f32)
    Wx = np.asarray(inputs["Wx"], f32)
    bx = np.asarray(inputs["bx"], f32)
    Wslice = np.asarray(inputs["Wslice"], f32)
    bslice = np.asarray(inputs["bslice"], f32)
    tau = np.asarray(inputs["temperature"], f32).reshape(HEADS)
    Wq = np.asarray(inputs["Wq"], f32)
    Wk = np.asarray(inputs["Wk"], f32)
    Wv = np.asarray(inputs["Wv"], f32)
    Wout = np.asarray(inputs["Wout"], f32)
    bout = np.asarray(inputs["bout"], f32)

    wlg_blocks = []
    blg_blocks = []
    for h in range(HEADS):
        Wx_h = Wx[h * DIM_HEAD:(h + 1) * DIM_HEAD, :]
        bx_h = bx[h * DIM_HEAD:(h + 1) * DIM_HEAD]
        wlg_blocks.append((Wslice @ Wx_h) / tau[h])
        blg_blocks.append((Wslice @ bx_h + bslice) / tau[h])
    wlgT = np.ascontiguousarray(np.concatenate(wlg_blocks, 0).T, f32)
    blg = np.concatenate(blg_blocks, 0).reshape(1, INNER).astype(f32)
    scale = DIM_HEAD ** -0.5
    return {
        "wfxT": np.ascontiguousarray(Wfx.T).astype(BF_NP),
        "wlgT": wlgT.astype(BF_NP),
        "blg": blg.astype(BF_NP),
        "onesb": np.ones((1, P), BF_NP),
        "bfxb": _bfx_pair(bfx),
        "wqT": np.ascontiguousarray((Wq * scale).T, f32),
        "wkT": np.ascontiguousarray(Wk.T, f32),
        "wvT": np.ascontiguousarray(Wv.T, f32),
        "woT": np.ascontiguousarray(
            Wout.T.reshape(HEADS, DIM_HEAD, DIM).transpose(1, 0, 2), f32),
        "bout8b": np.ascontiguousarray(
            np.tile(bout[None, None, :] / 8.0, (P, 2, 1)), f32),
        "idf32": np.eye(P, dtype=np.float32),
    }


_PROG = {}


def _get_prog(nshard, dbg=False):
    if (nshard, dbg) not in _PROG:
        _PROG[(nshard, dbg)] = build_program(nshard, dbg)
    return _PROG[(nshard, dbg)]


def run(inputs, nshard=NSHARD, trace=False, trace_cores=None, dbg=False):
    x = np.asarray(inputs["x"], np.float32)
    b_, n_, d_ = x.shape
    assert d_ == DIM and n_ == 2 * nshard and b_ == B
    nc = _get_prog(nshard, dbg)
    common = prep_weights(inputs)
    in_maps = []
    for core in range(NCORES):
        bb, half = core // 2, core % 2
        xs = x[bb, half * nshard:(half + 1) * nshard, :]
        m = dict(common)
        m["xT"] = np.ascontiguousarray(xs.T).astype(BF_NP)
        in_maps.append(m)
    res = run_bass_kernel_spmd(nc, in_maps, list(range(NCORES)),
                               trace=trace, trace_cores=trace_cores)
    full = np.empty((B, n_, DIM), np.float32)
    for core in range(NCORES):
        bb, half = core // 2, core % 2
        full[bb, half * nshard:(half + 1) * nshard, :] = \
            res.results[core]["out"]
    return full, res


def kernel(**inputs):
    out, _ = run(inputs)
    return out


# revision 44
# speedup vs baseline: 1.0071x; 1.0071x over previous
import sys, os
for _p in ("/opt/trn_rl_repo",):
    if _p not in sys.path:
        sys.path.append(_p)

import numpy as np
import ml_dtypes
from contextlib import ExitStack

import concourse.bass as bass
import concourse.bacc as bacc
import concourse.tile as tile
from concourse import mybir
from concourse.bass_utils import run_bass_kernel_spmd

F32 = mybir.dt.float32
BF16 = mybir.dt.bfloat16
BF_NP = ml_dtypes.bfloat16

DIM = 256
HEADS = 8
DIM_HEAD = 64
SLICE_NUM = 64
INNER = HEADS * DIM_HEAD  # 512
B, N = 4, 32768
NCORES = 8
NSHARD = N // 2  # 16384 tokens per core
P = 128
EXPF = mybir.ActivationFunctionType.Exp


def build_program(nshard, dbg=False):
    NT = nshard // P
    assert NT % 2 == 0
    nc = bacc.Bacc("TRN2", target_bir_lowering=False, debug=False,
                   num_devices=NCORES)
    if dbg:
        dbg_pooled = nc.dram_tensor("dbg_pooled", [P, 4, 65], F32,
                                    kind="ExternalOutput").ap()
        dbg_m2 = nc.dram_tensor("dbg_m2", [P, 4, DIM], BF16,
                                kind="ExternalOutput").ap()
        dbg_wT = nc.dram_tensor("dbg_wT", [P, 4, nshard], BF16,
                                kind="ExternalOutput").ap()
    xT_h = nc.dram_tensor("xT", [DIM, nshard], BF16, kind="ExternalInput")
    wfxT = nc.dram_tensor("wfxT", [DIM, INNER], BF16, kind="ExternalInput").ap()
    wlgT = nc.dram_tensor("wlgT", [DIM, INNER], BF16, kind="ExternalInput").ap()
    blg = nc.dram_tensor("blg", [1, INNER], BF16, kind="ExternalInput").ap()
    onesb = nc.dram_tensor("onesb", [1, P], BF16, kind="ExternalInput").ap()
    bfxb = nc.dram_tensor("bfxb", [P, 4, 64], F32, kind="ExternalInput").ap()
    wqT = nc.dram_tensor("wqT", [64, 64], F32, kind="ExternalInput").ap()
    wkT = nc.dram_tensor("wkT", [64, 64], F32, kind="ExternalInput").ap()
    wvT = nc.dram_tensor("wvT", [64, 64], F32, kind="ExternalInput").ap()
    woT = nc.dram_tensor("woT", [64, HEADS, DIM], F32, kind="ExternalInput").ap()
    bout8b = nc.dram_tensor("bout8b", [P, 2, DIM], F32,
                            kind="ExternalInput").ap()
    idf32 = nc.dram_tensor("idf32", [P, P], F32, kind="ExternalInput").ap()
    out_h = nc.dram_tensor("out", [nshard, DIM], F32, kind="ExternalOutput")
    out_ap = out_h.ap()

    with tile.TileContext(nc) as tc, ExitStack() as ctx:
        cpool = ctx.enter_context(tc.tile_pool(name="consts", bufs=1))
        big = ctx.enter_context(tc.tile_pool(name="big", bufs=1))

        # big weights on the scalar queue so x tiles start on sync at once
        wfx_sb = cpool.tile([P, 2, INNER], BF16)
        wlg_sb = cpool.tile([P, 2, INNER], BF16)
        for c in range(2):
            nc.scalar.dma_start(wfx_sb[:, c, :], wfxT[c * P:(c + 1) * P, :])
            nc.scalar.dma_start(wlg_sb[:, c, :], wlgT[c * P:(c + 1) * P, :])
        blg_sb = cpool.tile([1, INNER], BF16)
        nc.scalar.dma_start(blg_sb[:], blg[:])
        ones1 = cpool.tile([1, P], BF16)
        nc.scalar.dma_start(ones1[:], onesb[:])
        bfx_sb = cpool.tile([P, 4, 64], F32)
        nc.scalar.dma_start(bfx_sb[:], bfxb[:])
        wq_sb = cpool.tile([64, 64], F32)
        wk_sb = cpool.tile([64, 64], F32)
        wv_sb = cpool.tile([64, 64], F32)
        nc.scalar.dma_start(wq_sb[:], wqT[:])
        nc.scalar.dma_start(wk_sb[:], wkT[:])
        nc.scalar.dma_start(wv_sb[:], wvT[:])
        wo_sb = cpool.tile([64, HEADS, DIM], F32)
        nc.scalar.dma_start(wo_sb[:], woT[:])
        bout82_sb = cpool.tile([P, 2, DIM], F32)
        nc.scalar.dma_start(bout82_sb[:], bout8b[:])
        idf_sb = cpool.tile([P, P], F32)
        nc.scalar.dma_start(idf_sb[:], idf32[:])

        # persistent across phases
        # transposed slice weights, blocked: [g, t4, (t%4)*4+c, tok]
        wT_sb = big.tile([P, nshard // (4 * P), 16, P], BF16)
        pooled_sb = big.tile([P, 4, 65], F32)    # after allreduce (packed)
        m2_sb = big.tile([P, 4, DIM], BF16)      # out_slice @ WoutT per hg
        # manual 6-slot fx staging; ones cols preset once (norm columns)
        FXS = 6
        fx2_sb = big.tile([P, FXS, 4, 130], BF16)
        nc.vector.memset(fx2_sb[:, :, :, 128:130], 1.0)
        # 12-slot w staging ring, consumed by pools + 4-wide batched transpose
        WS = 12
        w8_sb = big.tile([P, WS, HEADS, SLICE_NUM], BF16)

        # ---------------- pass 1 ----------------
        # software-pipelined: pool matmuls + wT transpose for sub-tile t are
        # emitted DLY iterations late so the PE/sync queues never head-of-line
        # block on the exp->reduce->recip->mul chain.
        DLY = 4
        XB = 4  # sub-tiles per x DMA
        with tc.tile_pool(name="xp", bufs=3) as xpool, \
             tc.tile_pool(name="sp", bufs=8) as spool, \
             tc.tile_pool(name="fxps", bufs=3, space="PSUM") as fxps, \
             tc.tile_pool(name="lgps", bufs=3, space="PSUM") as lgps, \
             tc.tile_pool(name="poolps", bufs=1, space="PSUM") as poolps:
            # two accumulators per bank; start=True resets per-address, so
            # disjoint column ranges in one bank are safe
            pool_ps = [poolps.tile([P, 2, 130], F32, name=f"pool_ps{i}")
                       for i in range(2)]

            def emit_late(u):
                for q in range(4):
                    nc.tensor.matmul(pool_ps[q // 2][:, q % 2, :],
                                     w8_sb[:, u % WS, 2 * q:2 * q + 2, :],
                                     fx2_sb[:, u % FXS, q, :],
                                     start=(u == 0), stop=(u == NT - 1))
                if u % 4 == 3:
                    # one blocked DMA transpose for 4 sub-tiles:
                    # wT[g, (t',c), tok] = w[tok, (t',c)*128+g]
                    b = u // 4
                    s0 = (b % (WS // 4)) * 4
                    nc.sync.dma_start_transpose(
                        wT_sb[:, b, :, :], w8_sb[:, s0:s0 + 4, :, :])

            # warm the PE clock before the steady-state stream (scratch
            # writes into pool_ps[0]; the real accumulation's start=True at
            # t==0 overwrites them)
            for _ in range(15):
                nc.tensor.matmul(pool_ps[0][:], wfx_sb[:, 0, 0:128],
                                 wfx_sb[:, 0, 0:260], start=True, stop=True)

            for t in range(NT):
                if t % XB == 0:
                    xt = xpool.tile([P, 2, XB * P], BF16)
                    src = bass.AP(xT_h, t * P,
                                  [[nshard, P], [P * nshard, 2], [1, XB * P]])
                    nc.sync.dma_start(xt[:], src)
                s = t % XB
                xa = xt[:, 0, s * P:(s + 1) * P]
                xb = xt[:, 1, s * P:(s + 1) * P]
                fxp = fxps.tile([P, 4, P], F32)
                nc.tensor.matmul(fxp[:], xa, wfx_sb[:, 0, :],
                                 start=True, stop=False)
                nc.tensor.matmul(fxp[:], xb, wfx_sb[:, 1, :],
                                 start=False, stop=True)
                lgp = lgps.tile([P, HEADS, SLICE_NUM], F32)
                nc.tensor.matmul(lgp[:], ones1[:], blg_sb[:],
                                 start=True, stop=False)
                nc.tensor.matmul(lgp[:], xa, wlg_sb[:, 0, :],
                                 start=False, stop=False)
                nc.tensor.matmul(lgp[:], xb, wlg_sb[:, 1, :],
                                 start=False, stop=True)
                # softmax over slices (bounded logits: skip max-sub)
                nc.scalar.copy(fx2_sb[:, t % FXS, 0:2, 0:128], fxp[:, 0:2, :])
                e_t = spool.tile([P, HEADS, SLICE_NUM], BF16)
                nc.scalar.activation(e_t[:], lgp[:], EXPF)
                s_t = spool.tile([P, HEADS], F32)
                nc.vector.tensor_copy(fx2_sb[:, t % FXS, 2:4, 0:128],
                                      fxp[:, 2:4, :])
                nc.vector.tensor_reduce(s_t[:], e_t[:],
                                        axis=mybir.AxisListType.X,
                                        op=mybir.AluOpType.add)
                r_t = spool.tile([P, HEADS], F32)
                nc.vector.reciprocal(r_t[:], s_t[:])
                nc.gpsimd.tensor_mul(
                    w8_sb[:, t % WS, :, :], e_t[:],
                    r_t[:, :, None].to_broadcast([P, HEADS, SLICE_NUM]))
                if t >= DLY:
                    emit_late(t - DLY)
            for u in range(NT - DLY, NT):
                emit_late(u)

            # -------- allreduce pooled sums over the token-half pair --------
            # pack only what attention reads: diagonal block + norm col
            with tc.tile_pool(name="ccdram", bufs=1, space="DRAM") as dpool:
                b_in = dpool.tile([P, 4, 65], BF16)
                b_out = dpool.tile([P, 4, 65], BF16)
                pre_sb = big.tile([P, 4, 65], BF16)
                for i in range(2):
                    qs = slice(2 * i, 2 * i + 2)
                    nc.scalar.copy(pre_sb[0:64, qs, 0:64],
                                   pool_ps[i][0:64, :, 0:64])
                    nc.scalar.copy(pre_sb[0:64, qs, 64:65],
                                   pool_ps[i][0:64, :, 128:129])
                    nc.vector.tensor_copy(pre_sb[64:128, qs, :],
                                          pool_ps[i][64:128, :, 64:129])
                nc.sync.dma_start(b_in[:], pre_sb[:])
                nc.gpsimd.collective_compute(
                    "AllReduce", mybir.AluOpType.add,
                    replica_groups=[[0, 1], [2, 3], [4, 5], [6, 7]],
                    ins=[b_in.opt()], outs=[b_out.opt()])
                pooled_bf = big.tile([P, 4, 65], BF16)
                nc.sync.dma_start(pooled_bf[:], b_out[:])
                nc.vector.tensor_copy(pooled_sb[:], pooled_bf[:])

        # ------- tiny slice attention, stage-batched across head pairs ----
        with tc.tile_pool(name="mps", bufs=1, space="PSUM") as mps, \
             tc.tile_pool(name="msb", bufs=1) as msb:
            # slice tokens: st = (diag(pooled) + bfx*norm) / (norm + 1e-5)
            norm = pooled_sb[:, :, 64:65]
            nrm = msb.tile([P, 4], F32)
            nc.vector.tensor_scalar_add(nrm[:], pooled_sb[:, :, 64], 1e-5)
            rho = msb.tile([P, 4], F32)
            nc.vector.reciprocal(rho[:], nrm[:])
            tmp = msb.tile([P, 4, 64], F32)
            nc.vector.tensor_mul(tmp[:], bfx_sb[:],
                                 norm.to_broadcast([P, 4, 64]))
            nc.vector.tensor_add(tmp[:], tmp[:], pooled_sb[:, :, 0:64])
            st = msb.tile([P, 4, 64], F32)
            nc.vector.tensor_mul(st[:], tmp[:],
                                 rho[:, :, None].to_broadcast([P, 4, 64]))
            # stT[c, q4, j*64+g] = st[j*64+g, q4, c]
            pA = mps.tile([64, 4, P], F32, name="pA")  # stT / aT / osT
            pB = mps.tile([64, 4, P], F32, name="pB")  # q
            pC = mps.tile([64, 4, P], F32, name="pC")  # k
            pD = mps.tile([P, 4, 64], F32, name="pD")  # L / os
            pV0 = mps.tile([64, 4, 64], F32, name="pV0")
            pV1 = mps.tile([64, 4, 64], F32, name="pV1")
            pE = mps.tile([P, 2, DIM], F32, name="pE")
            pF = mps.tile([P, 2, DIM], F32, name="pF")
            for q4 in range(4):
                nc.tensor.transpose(pA[:, q4, :], st[:, q4, :], idf_sb[:])
            stT = msb.tile([64, 4, P], F32)
            nc.scalar.copy(stT[:], pA[:])
            for q4 in range(4):
                nc.tensor.matmul(pB[:, q4, :], wq_sb[:], stT[:, q4, :],
                                 start=True, stop=True)
                nc.tensor.matmul(pC[:, q4, :], wk_sb[:], stT[:, q4, :],
                                 start=True, stop=True)
            qq = msb.tile([64, 4, P], F32)
            nc.scalar.copy(qq[:], pB[:])
            kk = msb.tile([64, 4, P], F32)
            nc.vector.tensor_copy(kk[:], pC[:])
            # v[g', o] per (q4, j): lhsT = stT j-half, rhs = Wv
            for q4 in range(4):
                nc.tensor.matmul(pV0[:, q4, :], stT[:, q4, 0:64], wv_sb[:],
                                 start=True, stop=True)
                nc.tensor.matmul(pV1[:, q4, :], stT[:, q4, 64:128], wv_sb[:],
                                 start=True, stop=True)
            v0 = msb.tile([64, 4, 64], F32)
            nc.scalar.copy(v0[:], pV0[:])
            v1 = msb.tile([64, 4, 64], F32)
            nc.vector.tensor_copy(v1[:], pV1[:])
            # attention logits per (q4, head j)
            for q4 in range(4):
                for j in range(2):
                    sl = slice(j * 64, (j + 1) * 64)
                    nc.tensor.matmul(pD[sl, q4, :], qq[:, q4, sl],
                                     kk[:, q4, sl], start=True, stop=True)
            ea = msb.tile([P, 4, 64], F32)
            nc.scalar.activation(ea[:], pD[:], EXPF)
            sr = msb.tile([P, 4], F32)
            nc.vector.tensor_reduce(sr[:], ea[:], axis=mybir.AxisListType.X,
                                    op=mybir.AluOpType.add)
            rha = msb.tile([P, 4], F32)
            nc.vector.reciprocal(rha[:], sr[:])
            attn = msb.tile([P, 4, 64], F32)
            nc.vector.tensor_mul(attn[:], ea[:],
                                 rha[:, :, None].to_broadcast([P, 4, 64]))
            # aT[g, q4, j*64+g'] = attn[j*64+g', q4, g]
            for q4 in range(4):
                nc.tensor.transpose(pA[:, q4, :], attn[:, q4, :], idf_sb[:])
            aT = msb.tile([64, 4, P], F32)
            nc.scalar.copy(aT[:], pA[:])
            # os = attn @ v  (reuses pD)
            for q4 in range(4):
                for j in range(2):
                    sl = slice(j * 64, (j + 1) * 64)
                    nc.tensor.matmul(pD[sl, q4, :], aT[:, q4, sl],
                                     (v0 if j == 0 else v1)[:, q4, :],
                                     start=True, stop=True)
            os_sb = msb.tile([P, 4, 64], F32)
            nc.scalar.copy(os_sb[:], pD[:])
            for q4 in range(4):
                nc.tensor.transpose(pA[:, q4, :], os_sb[:, q4, :], idf_sb[:])
            osT = msb.tile([64, 4, P], F32)
            nc.scalar.copy(osT[:], pA[:])
            # m2 = osT @ WoutT per (q4, j), + bout/8 fold
            for q4 in range(4):
                pm = pE if q4 < 2 else pF
                for j in range(2):
                    sl = slice(j * 64, (j + 1) * 64)
                    nc.tensor.matmul(pm[sl, q4 % 2, :], osT[:, q4, sl],
                                     wo_sb[:, 2 * q4 + j, :],
                                     start=True, stop=True)
            nc.vector.tensor_add(m2_sb[:, 0:2, :], pE[:], bout82_sb[:])
            nc.vector.tensor_add(m2_sb[:, 2:4, :], pF[:], bout82_sb[:])

        # ---------------- pass 2: unpool + output proj ----------------
        with tc.tile_pool(name="p2ps", bufs=8, space="PSUM") as p2ps, \
             tc.tile_pool(name="p2sb", bufs=5) as p2sb:
            for t4 in range(NT // 4):
                ob4 = p2sb.tile([P, 4, DIM], F32)
                for k in range(4):
                    t = 4 * t4 + k
                    op = p2ps.tile([P, DIM], F32)
                    for c in range(4):
                        nc.tensor.matmul(
                            op[:], wT_sb[:, t4, k * 4 + c, :],
                            m2_sb[:, c, :],
                            start=(c == 0), stop=(c == 3))
                    if k % 2 == 0:
                        nc.vector.tensor_copy(ob4[:, k, :], op[:])
                    else:
                        nc.scalar.copy(ob4[:, k, :], op[:])
                # one quad DMA: dst rows t4*512 + k*128 + p
                dst = bass.AP(out_h, t4 * 4 * P * DIM,
                              [[DIM, P], [P * DIM, 4], [1, DIM]])
                nc.sync.dma_start(dst, ob4[:])
        if dbg:
            nc.sync.dma_start(dbg_pooled[:], pooled_sb[:])
            nc.sync.dma_start(dbg_m2[:], m2_sb[:])
            nc.sync.dma_start(dbg_wT[:], wT_sb[:])
    nc.compile()
    return nc


def _bfx_pair(bfx):
    bfx2 = bfx.reshape(HEADS, DIM_HEAD)
    out = np.empty((P, 4, 64), np.float32)
    for q4 in range(4):
        for j in range(2):
            out[j * 64:(j + 1) * 64, q4, :] = bfx2[2 * q4 + j]
    return out


def prep_weights(inputs):
    f32 = np.float32
    Wfx = np.asarray(inputs["Wfx"], f32)
    bfx = np.asarray(inputs["bfx"], f32)
    Wx = np.asarray(inputs["Wx"], f32)
    bx = np.asarray(inputs["bx"], f32)
    Wslice = np.asarray(inputs["Wslice"], f32)
    bslice = np.asarray(inputs["bslice"], f32)
    tau = np.asarray(inputs["temperature"], f32).reshape(HEADS)
    Wq = np.asarray(inputs["Wq"], f32)
    Wk = np.asarray(inputs["Wk"], f32)
    Wv = np.asarray(inputs["Wv"], f32)
    Wout = np.asarray(inputs["Wout"], f32)
    bout = np.asarray(inputs["bout"], f32)

    wlg_blocks = []
    blg_blocks = []
    for h in range(HEADS):
        Wx_h = Wx[h * DIM_HEAD:(h + 1) * DIM_HEAD, :]
        bx_h = bx[h * DIM_HEAD:(h + 1) * DIM_HEAD]
        wlg_blocks.append((Wslice @ Wx_h) / tau[h])
        blg_blocks.append((Wslice @ bx_h + bslice) / tau[h])
    wlgT = np.ascontiguousarray(np.concatenate(wlg_blocks, 0).T, f32)
    blg = np.concatenate(blg_blocks, 0).reshape(1, INNER).astype(f32)
    scale = DIM_HEAD ** -0.5
    return {
        "wfxT": np.ascontiguousarray(Wfx.T).astype(BF_NP),
        "wlgT": wlgT.astype(BF_NP),
        "blg": blg.astype(BF_NP),
        "onesb": np.ones((1, P), BF_NP),
        "bfxb": _bfx_pair(bfx),
        "wqT": np.ascontiguousarray((Wq * scale).T, f32),
        "wkT": np.ascontiguousarray(Wk.T, f32),
        "wvT": np.ascontiguousarray(Wv.T, f32),
        "woT": np.ascontiguousarray(
            Wout.T.reshape(HEADS, DIM_HEAD, DIM).transpose(1, 0, 2), f32),
        "bout8b": np.ascontiguousarray(
            np.tile(bout[None, None, :] / 8.0, (P, 2, 1)), f32),
        "idf32": np.eye(P, dtype=np.float32),
    }


_PROG = {}


def _get_prog(nshard, dbg=False):
    if (nshard, dbg) not in _PROG:
        _PROG[(nshard, dbg)] = build_program(nshard, dbg)
    return _PROG[(nshard, dbg)]


def run(inputs, nshard=NSHARD, trace=False, trace_cores=None, dbg=False):
    x = np.asarray(inputs["x"], np.float32)
    b_, n_, d_ = x.shape
    assert d_ == DIM and n_ == 2 * nshard and b_ == B
    nc = _get_prog(nshard, dbg)
    common = prep_weights(inputs)
    in_maps = []
    for core in range(NCORES):
        bb, half = core // 2, core % 2
        xs = x[bb, half * nshard:(half + 1) * nshard, :]
        m = dict(common)
        m["xT"] = np.ascontiguousarray(xs.T).astype(BF_NP)
        in_maps.append(m)
    res = run_bass_kernel_spmd(nc, in_maps, list(range(NCORES)),
                               trace=trace, trace_cores=trace_cores)
    full = np.empty((B, n_, DIM), np.float32)
    for core in range(NCORES):
        bb, half = core // 2, core % 2
        full[bb, half * nshard:(half + 1) * nshard, :] = \
            res.results[core]["out"]
    return full, res


def kernel(**inputs):
    out, _ = run(inputs)
    return out


# revision 46
# speedup vs baseline: 1.1331x; 1.1252x over previous
import sys, os
for _p in ("/opt/trn_rl_repo",):
    if _p not in sys.path:
        sys.path.append(_p)

import numpy as np
import ml_dtypes
from contextlib import ExitStack

import concourse.bass as bass
import concourse.bacc as bacc
import concourse.tile as tile
from concourse import mybir
from concourse.bass_utils import run_bass_kernel_spmd

F32 = mybir.dt.float32
BF16 = mybir.dt.bfloat16
BF_NP = ml_dtypes.bfloat16

DIM = 256
HEADS = 8
DIM_HEAD = 64
SLICE_NUM = 64
INNER = HEADS * DIM_HEAD  # 512
B, N = 4, 32768
NCORES = 8
NSHARD = N // 2  # 16384 tokens per core
P = 128
EXPF = mybir.ActivationFunctionType.Exp


def build_program(nshard, dbg=False):
    NT = nshard // P
    assert NT % 2 == 0
    nc = bacc.Bacc("TRN2", target_bir_lowering=False, debug=False,
                   num_devices=NCORES)
    if dbg:
        dbg_pooled = nc.dram_tensor("dbg_pooled", [P, 4, 65], F32,
                                    kind="ExternalOutput").ap()
        dbg_m2 = nc.dram_tensor("dbg_m2", [P, 4, DIM], BF16,
                                kind="ExternalOutput").ap()
        dbg_wT = nc.dram_tensor("dbg_wT", [P, 4, nshard], BF16,
                                kind="ExternalOutput").ap()
    xT_h = nc.dram_tensor("xT", [DIM, nshard], BF16, kind="ExternalInput")
    wfxT = nc.dram_tensor("wfxT", [DIM, INNER], BF16, kind="ExternalInput").ap()
    wlgT = nc.dram_tensor("wlgT", [DIM, INNER], BF16, kind="ExternalInput").ap()
    blg = nc.dram_tensor("blg", [1, INNER], BF16, kind="ExternalInput").ap()
    onesb = nc.dram_tensor("onesb", [1, P], BF16, kind="ExternalInput").ap()
    bfxb = nc.dram_tensor("bfxb", [P, 4, 64], F32, kind="ExternalInput").ap()
    wqT = nc.dram_tensor("wqT", [64, 64], F32, kind="ExternalInput").ap()
    wkT = nc.dram_tensor("wkT", [64, 64], F32, kind="ExternalInput").ap()
    wvT = nc.dram_tensor("wvT", [64, 64], F32, kind="ExternalInput").ap()
    woT = nc.dram_tensor("woT", [64, HEADS, DIM], F32, kind="ExternalInput").ap()
    bout8b = nc.dram_tensor("bout8b", [P, 2, DIM], F32,
                            kind="ExternalInput").ap()
    idf32 = nc.dram_tensor("idf32", [P, P], F32, kind="ExternalInput").ap()
    out_h = nc.dram_tensor("out", [nshard, DIM], F32, kind="ExternalOutput")
    out_ap = out_h.ap()

    with tile.TileContext(nc) as tc, ExitStack() as ctx:
        cpool = ctx.enter_context(tc.tile_pool(name="consts", bufs=1))
        big = ctx.enter_context(tc.tile_pool(name="big", bufs=1))

        # big weights on the scalar queue so x tiles start on sync at once
        wfx_sb = cpool.tile([P, 2, INNER], BF16)
        wlg_sb = cpool.tile([P, 2, INNER], BF16)
        for c in range(2):
            nc.scalar.dma_start(wfx_sb[:, c, :], wfxT[c * P:(c + 1) * P, :])
            nc.scalar.dma_start(wlg_sb[:, c, :], wlgT[c * P:(c + 1) * P, :])
        blg_sb = cpool.tile([1, INNER], BF16)
        nc.scalar.dma_start(blg_sb[:], blg[:])
        ones1 = cpool.tile([1, P], BF16)
        nc.scalar.dma_start(ones1[:], onesb[:])
        bfx_sb = cpool.tile([P, 4, 64], F32)
        nc.scalar.dma_start(bfx_sb[:], bfxb[:])
        wq_sb = cpool.tile([64, 64], F32)
        wk_sb = cpool.tile([64, 64], F32)
        wv_sb = cpool.tile([64, 64], F32)
        nc.scalar.dma_start(wq_sb[:], wqT[:])
        nc.scalar.dma_start(wk_sb[:], wkT[:])
        nc.scalar.dma_start(wv_sb[:], wvT[:])
        wo_sb = cpool.tile([64, HEADS, DIM], F32)
        nc.scalar.dma_start(wo_sb[:], woT[:])
        bout82_sb = cpool.tile([P, 2, DIM], F32)
        nc.scalar.dma_start(bout82_sb[:], bout8b[:])
        idf_sb = cpool.tile([P, P], F32)
        nc.scalar.dma_start(idf_sb[:], idf32[:])

        # persistent across phases
        # transposed slice weights, blocked: [g, t4, (t%4)*4+c, tok]
        wT_sb = big.tile([P, nshard // (4 * P), 16, P], BF16)
        pooled_sb = big.tile([P, 4, 65], F32)    # after allreduce (packed)
        m2_sb = big.tile([P, 4, DIM], BF16)      # out_slice @ WoutT per hg
        # manual 6-slot fx staging; ones cols preset once (norm columns)
        FXS = 6
        fx2_sb = big.tile([P, FXS, 4, 130], BF16)
        nc.vector.memset(fx2_sb[:, :, :, 128:130], 1.0)
        # 12-slot w staging ring, consumed by pools + 4-wide batched transpose
        WS = 12
        w8_sb = big.tile([P, WS, HEADS, SLICE_NUM], BF16)

        # ---------------- pass 1 ----------------
        # software-pipelined: pool matmuls + wT transpose for sub-tile t are
        # emitted DLY iterations late so the PE/sync queues never head-of-line
        # block on the exp->reduce->recip->mul chain.
        DLY = 4
        XB = 8  # sub-tiles per x DMA
        with tc.tile_pool(name="xp", bufs=3) as xpool, \
             tc.tile_pool(name="sp", bufs=8) as spool, \
             tc.tile_pool(name="fxps", bufs=3, space="PSUM") as fxps, \
             tc.tile_pool(name="lgps", bufs=3, space="PSUM") as lgps, \
             tc.tile_pool(name="poolps", bufs=1, space="PSUM") as poolps:
            # two accumulators per bank; start=True resets per-address, so
            # disjoint column ranges in one bank are safe
            pool_ps = [poolps.tile([P, 2, 130], F32, name=f"pool_ps{i}")
                       for i in range(2)]

            def emit_late(u):
                for q in range(4):
                    nc.tensor.matmul(pool_ps[q // 2][:, q % 2, :],
                                     w8_sb[:, u % WS, 2 * q:2 * q + 2, :],
                                     fx2_sb[:, u % FXS, q, :],
                                     start=(u == 0), stop=(u == NT - 1))
                if u % 4 == 3:
                    # one blocked DMA transpose for 4 sub-tiles:
                    # wT[g, (t',c), tok] = w[tok, (t',c)*128+g]
                    b = u // 4
                    s0 = (b % (WS // 4)) * 4
                    nc.sync.dma_start_transpose(
                        wT_sb[:, b, :, :], w8_sb[:, s0:s0 + 4, :, :])

            # warm the PE clock before the steady-state stream (scratch
            # writes into pool_ps[0]; the real accumulation's start=True at
            # t==0 overwrites them)
            for _ in range(15):
                nc.tensor.matmul(pool_ps[0][:], wfx_sb[:, 0, 0:128],
                                 wfx_sb[:, 0, 0:260], start=True, stop=True)

            for t in range(NT):
                if t % XB == 0:
                    xt = xpool.tile([P, 2, XB * P], BF16)
                    src = bass.AP(xT_h, t * P,
                                  [[nshard, P], [P * nshard, 2], [1, XB * P]])
                    nc.sync.dma_start(xt[:], src)
                s = t % XB
                xa = xt[:, 0, s * P:(s + 1) * P]
                xb = xt[:, 1, s * P:(s + 1) * P]
                fxp = fxps.tile([P, 4, P], F32)
                nc.tensor.matmul(fxp[:], xa, wfx_sb[:, 0, :],
                                 start=True, stop=False)
                nc.tensor.matmul(fxp[:], xb, wfx_sb[:, 1, :],
                                 start=False, stop=True)
                lgp = lgps.tile([P, HEADS, SLICE_NUM], F32)
                nc.tensor.matmul(lgp[:], ones1[:], blg_sb[:],
                                 start=True, stop=False)
                nc.tensor.matmul(lgp[:], xa, wlg_sb[:, 0, :],
                                 start=False, stop=False)
                nc.tensor.matmul(lgp[:], xb, wlg_sb[:, 1, :],
                                 start=False, stop=True)
                # softmax over slices (bounded logits: skip max-sub)
                nc.scalar.copy(fx2_sb[:, t % FXS, 0:2, 0:128], fxp[:, 0:2, :])
                e_t = spool.tile([P, HEADS, SLICE_NUM], BF16)
                nc.scalar.activation(e_t[:], lgp[:], EXPF)
                s_t = spool.tile([P, HEADS], F32)
                nc.vector.tensor_copy(fx2_sb[:, t % FXS, 2:4, 0:128],
                                      fxp[:, 2:4, :])
                nc.vector.tensor_reduce(s_t[:], e_t[:],
                                        axis=mybir.AxisListType.X,
                                        op=mybir.AluOpType.add)
                r_t = spool.tile([P, HEADS], F32)
                nc.vector.reciprocal(r_t[:], s_t[:])
                nc.gpsimd.tensor_mul(
                    w8_sb[:, t % WS, :, :], e_t[:],
                    r_t[:, :, None].to_broadcast([P, HEADS, SLICE_NUM]))
                if t >= DLY:
                    emit_late(t - DLY)
            for u in range(NT - DLY, NT):
                emit_late(u)

            # -------- allreduce pooled sums over the token-half pair --------
            # pack only what attention reads: diagonal block + norm col
            with tc.tile_pool(name="ccdram", bufs=1, space="DRAM") as dpool:
                b_in = dpool.tile([P, 4, 65], BF16)
                b_out = dpool.tile([P, 4, 65], BF16)
                pre_sb = big.tile([P, 4, 65], BF16)
                for i in range(2):
                    qs = slice(2 * i, 2 * i + 2)
                    nc.scalar.copy(pre_sb[0:64, qs, 0:64],
                                   pool_ps[i][0:64, :, 0:64])
                    nc.scalar.copy(pre_sb[0:64, qs, 64:65],
                                   pool_ps[i][0:64, :, 128:129])
                    nc.vector.tensor_copy(pre_sb[64:128, qs, :],
                                          pool_ps[i][64:128, :, 64:129])
                nc.sync.dma_start(b_in[:], pre_sb[:])
                nc.gpsimd.collective_compute(
                    "AllReduce", mybir.AluOpType.add,
                    replica_groups=[[0, 1], [2, 3], [4, 5], [6, 7]],
                    ins=[b_in.opt()], outs=[b_out.opt()])
                pooled_bf = big.tile([P, 4, 65], BF16)
                nc.sync.dma_start(pooled_bf[:], b_out[:])
                nc.vector.tensor_copy(pooled_sb[:], pooled_bf[:])

        # ------- tiny slice attention, stage-batched across head pairs ----
        with tc.tile_pool(name="mps", bufs=1, space="PSUM") as mps, \
             tc.tile_pool(name="msb", bufs=1) as msb:
            # slice tokens: st = (diag(pooled) + bfx*norm) / (norm + 1e-5)
            norm = pooled_sb[:, :, 64:65]
            nrm = msb.tile([P, 4], F32)
            nc.vector.tensor_scalar_add(nrm[:], pooled_sb[:, :, 64], 1e-5)
            rho = msb.tile([P, 4], F32)
            nc.vector.reciprocal(rho[:], nrm[:])
            tmp = msb.tile([P, 4, 64], F32)
            nc.vector.tensor_mul(tmp[:], bfx_sb[:],
                                 norm.to_broadcast([P, 4, 64]))
            nc.vector.tensor_add(tmp[:], tmp[:], pooled_sb[:, :, 0:64])
            st = msb.tile([P, 4, 64], F32)
            nc.vector.tensor_mul(st[:], tmp[:],
                                 rho[:, :, None].to_broadcast([P, 4, 64]))
            # stT[c, q4, j*64+g] = st[j*64+g, q4, c]
            pA = mps.tile([64, 4, P], F32, name="pA")  # stT / aT / osT
            pB = mps.tile([64, 4, P], F32, name="pB")  # q
            pC = mps.tile([64, 4, P], F32, name="pC")  # k
            pD = mps.tile([P, 4, 64], F32, name="pD")  # L / os
            pV0 = mps.tile([64, 4, 64], F32, name="pV0")
            pV1 = mps.tile([64, 4, 64], F32, name="pV1")
            pE = mps.tile([P, 2, DIM], F32, name="pE")
            pF = mps.tile([P, 2, DIM], F32, name="pF")
            for q4 in range(4):
                nc.tensor.transpose(pA[:, q4, :], st[:, q4, :], idf_sb[:])
            stT = msb.tile([64, 4, P], F32)
            nc.scalar.copy(stT[:], pA[:])
            for q4 in range(4):
                nc.tensor.matmul(pB[:, q4, :], wq_sb[:], stT[:, q4, :],
                                 start=True, stop=True)
                nc.tensor.matmul(pC[:, q4, :], wk_sb[:], stT[:, q4, :],
                                 start=True, stop=True)
            qq = msb.tile([64, 4, P], F32)
            nc.scalar.copy(qq[:], pB[:])
            kk = msb.tile([64, 4, P], F32)
            nc.vector.tensor_copy(kk[:], pC[:])
            # v[g', o] per (q4, j): lhsT = stT j-half, rhs = Wv
            for q4 in range(4):
                nc.tensor.matmul(pV0[:, q4, :], stT[:, q4, 0:64], wv_sb[:],
                                 start=True, stop=True)
                nc.tensor.matmul(pV1[:, q4, :], stT[:, q4, 64:128], wv_sb[:],
                                 start=True, stop=True)
            v0 = msb.tile([64, 4, 64], F32)
            nc.scalar.copy(v0[:], pV0[:])
            v1 = msb.tile([64, 4, 64], F32)
            nc.vector.tensor_copy(v1[:], pV1[:])
            # attention logits per (q4, head j)
            for q4 in range(4):
                for j in range(2):
                    sl = slice(j * 64, (j + 1) * 64)
                    nc.tensor.matmul(pD[sl, q4, :], qq[:, q4, sl],
                                     kk[:, q4, sl], start=True, stop=True)
            ea = msb.tile([P, 4, 64], F32)
            nc.scalar.activation(ea[:], pD[:], EXPF)
            sr = msb.tile([P, 4], F32)
            nc.vector.tensor_reduce(sr[:], ea[:], axis=mybir.AxisListType.X,
                                    op=mybir.AluOpType.add)
            rha = msb.tile([P, 4], F32)
            nc.vector.reciprocal(rha[:], sr[:])
            attn = msb.tile([P, 4, 64], F32)
            nc.vector.tensor_mul(attn[:], ea[:],
                                 rha[:, :, None].to_broadcast([P, 4, 64]))
            # aT[g, q4, j*64+g'] = attn[j*64+g', q4, g]
            for q4 in range(4):
                nc.tensor.transpose(pA[:, q4, :], attn[:, q4, :], idf_sb[:])
            aT = msb.tile([64, 4, P], F32)
            nc.scalar.copy(aT[:], pA[:])
            # os = attn @ v  (reuses pD)
            for q4 in range(4):
                for j in range(2):
                    sl = slice(j * 64, (j + 1) * 64)
                    nc.tensor.matmul(pD[sl, q4, :], aT[:, q4, sl],
                                     (v0 if j == 0 else v1)[:, q4, :],
                                     start=True, stop=True)
            os_sb = msb.tile([P, 4, 64], F32)
            nc.scalar.copy(os_sb[:], pD[:])
            for q4 in range(4):
                nc.tensor.transpose(pA[:, q4, :], os_sb[:, q4, :], idf_sb[:])
            osT = msb.tile([64, 4, P], F32)
            nc.scalar.copy(osT[:], pA[:])
            # m2 = osT @ WoutT per (q4, j), + bout/8 fold
            for q4 in range(4):
                pm = pE if q4 < 2 else pF
                for j in range(2):
                    sl = slice(j * 64, (j + 1) * 64)
                    nc.tensor.matmul(pm[sl, q4 % 2, :], osT[:, q4, sl],
                                     wo_sb[:, 2 * q4 + j, :],
                                     start=True, stop=True)
            nc.vector.tensor_add(m2_sb[:, 0:2, :], pE[:], bout82_sb[:])
            nc.vector.tensor_add(m2_sb[:, 2:4, :], pF[:], bout82_sb[:])

        # ---------------- pass 2: unpool + output proj ----------------
        with tc.tile_pool(name="p2ps", bufs=8, space="PSUM") as p2ps, \
             tc.tile_pool(name="p2sb", bufs=5) as p2sb:
            for t4 in range(NT // 4):
                ob4 = p2sb.tile([P, 4, DIM], F32)
                for k in range(4):
                    t = 4 * t4 + k
                    op = p2ps.tile([P, DIM], F32)
                    for c in range(4):
                        nc.tensor.matmul(
                            op[:], wT_sb[:, t4, k * 4 + c, :],
                            m2_sb[:, c, :],
                            start=(c == 0), stop=(c == 3))
                    if k % 2 == 0:
                        nc.vector.tensor_copy(ob4[:, k, :], op[:])
                    else:
                        nc.scalar.copy(ob4[:, k, :], op[:])
                # one quad DMA: dst rows t4*512 + k*128 + p
                dst = bass.AP(out_h, t4 * 4 * P * DIM,
                              [[DIM, P], [P * DIM, 4], [1, DIM]])
                if t4 % 2 == 0:
                    nc.sync.dma_start(dst, ob4[:])
                else:
                    nc.scalar.dma_start(dst, ob4[:])
        if dbg:
            nc.sync.dma_start(dbg_pooled[:], pooled_sb[:])
            nc.sync.dma_start(dbg_m2[:], m2_sb[:])
            nc.sync.dma_start(dbg_wT[:], wT_sb[:])
    nc.compile()
    return nc


def _bfx_pair(bfx):
    bfx2 = bfx.reshape(HEADS, DIM_HEAD)
    out = np.empty((P, 4, 64), np.float32)
    for q4 in range(4):
        for j in range(2):
            out[j * 64:(j + 1) * 64, q4, :] = bfx2[2 * q4 + j]
    return out


def prep_weights(inputs):
    f32 = np.float32
    Wfx = np.asarray(inputs["Wfx"], f32)
    bfx = np.asarray(inputs["bfx"], f32)
    Wx = np.asarray(inputs["Wx"], f32)
    bx = np.asarray(inputs["bx"], f32)
    Wslice = np.asarray(inputs["Wslice"], f32)
    bslice = np.asarray(inputs["bslice"], f32)
    tau = np.asarray(inputs["temperature"], f32).reshape(HEADS)
    Wq = np.asarray(inputs["Wq"], f32)
    Wk = np.asarray(inputs["Wk"], f32)
    Wv = np.asarray(inputs["Wv"], f32)
    Wout = np.asarray(inputs["Wout"], f32)
    bout = np.asarray(inputs["bout"], f32)

    wlg_blocks = []
    blg_blocks = []
    for h in range(HEADS):
        Wx_h = Wx[h * DIM_HEAD:(h + 1) * DIM_HEAD, :]
        bx_h = bx[h * DIM_HEAD:(h + 1) * DIM_HEAD]
        wlg_blocks.append((Wslice @ Wx_h) / tau[h])
        blg_blocks.append((Wslice @ bx_h + bslice) / tau[h])
    wlgT = np.ascontiguousarray(np.concatenate(wlg_blocks, 0).T, f32)
    blg = np.concatenate(blg_blocks, 0).reshape(1, INNER).astype(f32)
    scale = DIM_HEAD ** -0.5
    return {
        "wfxT": np.ascontiguousarray(Wfx.T).astype(BF_NP),
        "wlgT": wlgT.astype(BF_NP),
        "blg": blg.astype(BF_NP),
        "onesb": np.ones((1, P), BF_NP),
        "bfxb": _bfx_pair(bfx),
        "wqT": np.ascontiguousarray((Wq * scale).T, f32),
        "wkT": np.ascontiguousarray(Wk.T, f32),
        "wvT": np.ascontiguousarray(Wv.T, f32),
        "woT": np.ascontiguousarray(
            Wout.T.reshape(HEADS, DIM_HEAD, DIM).transpose(1, 0, 2), f32),
        "bout8b": np.ascontiguousarray(
            np.tile(bout[None, None, :] / 8.0, (P, 2, 1)), f32),
        "idf32": np.eye(P, dtype=np.float32),
    }


_PROG = {}


def _get_prog(nshard, dbg=False):
    if (nshard, dbg) not in _PROG:
        _PROG[(nshard, dbg)] = build_program(nshard, dbg)
    return _PROG[(nshard, dbg)]


def run(inputs, nshard=NSHARD, trace=False, trace_cores=None, dbg=False):
    x = np.asarray(inputs["x"], np.float32)
    b_, n_, d_ = x.shape
    assert d_ == DIM and n_ == 2 * nshard and b_ == B
    nc = _get_prog(nshard, dbg)
    common = prep_weights(inputs)
    in_maps = []
    for core in range(NCORES):
        bb, half = core // 2, core % 2
        xs = x[bb, half * nshard:(half + 1) * nshard, :]
        m = dict(common)
        m["xT"] = np.ascontiguousarray(xs.T).astype(BF_NP)
        in_maps.append(m)
    res = run_bass_kernel_spmd(nc, in_maps, list(range(NCORES)),
                               trace=trace, trace_cores=trace_cores)
    full = np.empty((B, n_, DIM), np.float32)
    for core in range(NCORES):
        bb, half = core // 2, core % 2
        full[bb, half * nshard:(half + 1) * nshard, :] = \
            res.results[core]["out"]
    return full, res


def kernel(**inputs):
    out, _ = run(inputs)
    return out
